# revision 29
# baseline (speedup 1.0000x reference)
"""Trainium2 Bass kernel for agglomerative pairwise conv-merging (retrieval_knn).

Problem: x (16,16,512,8,8) f32. Per batch element: n=16 slots, 15 merge
iterations; each picks the argmin pair of the masked pairwise squared-distance
matrix, merges the pair via a 2->1-channel 7x7 conv (pad 3), writes the result
to a new slot. Output = final slot per batch element.

Strategy: pure data parallelism, 2 batch elements per core on 8 cores; the
whole per-core working set lives in SBUF so HBM traffic is one input read +
one output write. The two batch elements share no state, so the Tile
scheduler can overlap one batch's TensorE phase with the other's VectorE
phase. Per merge iteration, fully on-chip:
  - cf-major slot layout (128, cf, s) per batch: partition = (c%2)*64 + h*8+w,
    cf = c//2. An f32r (FP22) shadow feeds all distance matmuls at
    1 cycle/row; the fp32 master feeds the conv and the output.
  - conv == per-channel 64x64 Toeplitz matmul (pad-3 7x7 conv on 8x8 is a
    dense 64x64 spatial operator); block-diag 128x128 on the PE, reading the
    merging pair in place through register-indexed dynamic APs.
  - negated masked distances live free-form on partition 0 as 32-padded
    matrices, so argmin is a flat max/max_index and row/col kills are
    free-dim memsets with register-derived offsets (i = idx>>5, j = idx&31).
  - new-slot dots vs all slots: 16 cf-chunks per f32r matmul (N=496,
    16 matmuls per batch), then diag-block extraction matmuls accumulated
    in PSUM.
"""

import sys

sys.path.insert(0, "/opt/trn_rl_repo")

import numpy as np

import concourse.bacc as bacc
import concourse.bass as bass
import concourse.mybir as mybir
from concourse.bass_utils import run_bass_kernel_spmd
from concourse.tile import TileContext

dt = mybir.dt
Alu = mybir.AluOpType
DVE = mybir.EngineType.DVE
PE = mybir.EngineType.PE
ACT_IDENT = mybir.ActivationFunctionType.Identity
ds = bass.ds

N_CORES = 8
N0 = 16          # initial slots per batch element
M = 31           # total slots (2*16-1)
CF = 256         # channel pairs (512 channels / 2)
NEG = -1e30
USE_DYN_CONV = True


def _build_A(w7: np.ndarray) -> np.ndarray:
    """(7,7) conv weights -> dense (64,64) operator on the 8x8 spatial dim."""
    A = np.zeros((64, 64), np.float32)
    ky, kx = np.meshgrid(np.arange(7), np.arange(7), indexing="ij")
    for y in range(8):
        for x in range(8):
            yy, xx = y + ky - 3, x + kx - 3
            ok = (yy >= 0) & (yy < 8) & (xx >= 0) & (xx < 8)
            A[y * 8 + x, (yy * 8 + xx)[ok]] = w7[ok]
    return A


def _build_program():
    nc = bacc.Bacc(None, target_bir_lowering=False, debug=False)
    xpf = nc.declare_dram_parameter("xpf", [128, 2 * CF * M], dt.float32, isOutput=False)
    alT = nc.declare_dram_parameter("alT", [128, 128], dt.float32, isOutput=False)
    arT = nc.declare_dram_parameter("arT", [128, 128], dt.float32, isOutput=False)
    biasd = nc.declare_dram_parameter("biasd", [128, 1], dt.float32, isOutput=False)
    id64d = nc.declare_dram_parameter("id64d", [64, 64], dt.float32, isOutput=False)
    y = nc.declare_dram_parameter("y", [128, 2, CF], dt.float32, isOutput=True)

    with TileContext(nc) as tc:
        with (
            tc.tile_pool(name="persist", bufs=1) as pp,
            tc.tile_pool(name="scratch", bufs=4) as sp,
            tc.tile_pool(name="ps0", bufs=1, space="PSUM") as ps0,
            tc.tile_pool(name="ps1", bufs=1, space="PSUM") as ps1,
        ):
            # ---- persistent state (independent per batch) ----
            xbs = [pp.tile([128, CF, M], dt.float32, tag=f"xb{b}", name=f"xb{b}") for b in (0, 1)]
            xbrs = [pp.tile([128, CF, 32], dt.float32r, tag=f"xbr{b}", name=f"xbr{b}") for b in (0, 1)]
            mstks = [pp.tile([128, CF], dt.float32r, tag=f"mstk{b}", name=f"mstk{b}") for b in (0, 1)]
            ndmat = pp.tile([1, 2048], dt.float32, tag="ndmat")
            pmaskF = pp.tile([1, 2 * M], dt.float32, tag="pmaskF")
            sq = pp.tile([1, 2 * M], dt.float32, tag="sq")
            gflat = pp.tile([1, 2, 16, 16], dt.float32, tag="gflat")
            al_sb = pp.tile([128, 128], dt.float32, tag="al")
            ar_sb = pp.tile([128, 128], dt.float32, tag="ar")
            bias_sb = pp.tile([128, 1], dt.float32, tag="bias")
            id64 = pp.tile([64, 64], dt.float32, tag="id64")

            xbrFs = [t.rearrange("p f s -> p (f s)") for t in xbrs]
            v4 = ndmat.rearrange("p (b r c) -> p b r c", b=2, r=32)

            # ---- loads ----
            nc.sync.dma_start(al_sb, alT[:])
            nc.sync.dma_start(ar_sb, arT[:])
            nc.sync.dma_start(bias_sb, biasd[:])
            nc.sync.dma_start(id64, id64d[:])
            for b in (0, 1):
                nc.sync.dma_start(xbs[b].rearrange("p f s -> p (f s)"),
                                  xpf[:, b * CF * M:(b + 1) * CF * M])
                nc.vector.tensor_copy(xbrs[b][:, :, 0:M], xbs[b])
                nc.vector.tensor_copy(xbrs[b][:, :, M:32], xbs[b][:, :, 0:1])
            nc.vector.memset(ndmat, NEG)
            nc.vector.memset(pmaskF, NEG)
            nc.vector.memset(sq, 0.0)
            for b in (0, 1):
                nc.vector.memset(pmaskF[0:1, M * b:M * b + N0], 0.0)

            # ---- initial Gram per batch (f32r; FP22 is plenty for argmin) ----
            for b in (0, 1):
                psp = ps0 if b == 0 else ps1
                pG = psp.tile([32, 32], dt.float32, tag=f"pd{b}")
                for cf in range(CF):
                    src = xbrs[b][:, cf, :]
                    nc.tensor.matmul(pG, src, src,
                                     start=(cf == 0), stop=(cf == CF - 1))
                gsb = sp.tile([32, 32], dt.float32, tag=f"gsb{b}")
                nc.vector.tensor_copy(gsb, pG)
                # flatten the 16 needed Gram rows to free-form
                for k in range(16):
                    pr = psp.tile([1, 16], dt.float32, tag=f"pdd{b}")
                    nc.tensor.matmul(pr, id64[0:32, k:k + 1], gsb[:, 0:16],
                                     start=True, stop=True)
                    nc.vector.tensor_copy(gflat[0:1, b, k, :], pr)
                for k in range(16):
                    nc.vector.tensor_copy(sq[0:1, M * b + k:M * b + k + 1],
                                          gflat[0:1, b, k, k:k + 1])
                # initial negated distances: nd[k,s] = 2 G[k,s] - sq_k - sq_s
                for k in range(16):
                    t = sp.tile([1, 16], dt.float32, tag=f"tinit{b}")
                    nc.vector.tensor_scalar(
                        t, gflat[0:1, b, k, :], 2.0,
                        sq[0:1, M * b + k:M * b + k + 1], Alu.mult, Alu.subtract)
                    nc.vector.tensor_sub(v4[0:1, b, k, 0:16], t,
                                         sq[0:1, M * b:M * b + 16])
                    nc.vector.memset(v4[0:1, b, k, k:k + 1], NEG)

            # ---- 15 merge iterations ----
            for it in range(M - N0):
                slot = N0 + it
                last = it == M - N0 - 1
                for b in (0, 1):
                    psp = ps0 if b == 0 else ps1
                    xb, xbr, mstk, xbrF = xbs[b], xbrs[b], mstks[b], xbrFs[b]

                    # argmin over masked negated distances
                    mx8 = sp.tile([1, 8], dt.float32, tag=f"mx8_{b}")
                    ix8 = sp.tile([1, 8], dt.uint32, tag=f"ix8_{b}")
                    nd_b = ndmat[0:1, 1024 * b:1024 * (b + 1)]
                    nc.vector.max(mx8, nd_b)
                    nc.vector.max_index(ix8, mx8, nd_b)
                    engines = (DVE, PE) if USE_DYN_CONV else (DVE,)
                    idx_sv = nc.values_load(ix8[0:1, 0:1], engines=engines,
                                            min_val=0, max_val=1023,
                                            skip_runtime_bounds_check=True)
                    ri = nc.alloc_registers(f"ri{it}_{b}", engines=engines)
                    nc.regs_alu(ri, idx_sv, 5, Alu.logical_shift_right)
                    i_sv = nc.snap(ri, donate=True, min_val=0, max_val=M - 1)
                    rj = nc.alloc_registers(f"rj{it}_{b}", engines=engines)
                    nc.regs_alu(rj, idx_sv, 31, Alu.bitwise_and)
                    j_sv = nc.snap(rj, donate=True, min_val=0, max_val=M - 1)

                    # alive-mask + row/col kills (before the new row is built)
                    pmb = pmaskF[0:1, M * b:M * b + M]
                    nc.vector.memset(pmb[0:1, ds(i_sv, 1)], NEG)
                    nc.vector.memset(pmb[0:1, ds(j_sv, 1)], NEG)
                    if not last:
                        nc.vector.memset(v4[0:1, b, ds(i_sv, 1), :], NEG)
                        nc.vector.memset(v4[0:1, b, ds(j_sv, 1), :], NEG)
                        nc.vector.memset(v4[0:1, b, 0:M, ds(i_sv, 1)], NEG)
                        nc.vector.memset(v4[0:1, b, 0:M, ds(j_sv, 1)], NEG)
                        nc.vector.memset(pmaskF[0:1, M * b + slot:M * b + slot + 1], 0.0)

                    # conv merge m = A_l @ x_i + A_r @ x_j + bias
                    pcv = psp.tile([128, CF], dt.float32, tag=f"pcv{b}")
                    if USE_DYN_CONV:
                        nc.tensor.matmul(pcv, al_sb, xb[:, :, ds(i_sv, 1)],
                                         start=True, stop=False)
                        nc.tensor.matmul(pcv, ar_sb, xb[:, :, ds(j_sv, 1)],
                                         start=False, stop=True)
                    else:
                        xl = sp.tile([128, CF], dt.float32, tag=f"xl{b}")
                        xr = sp.tile([128, CF], dt.float32, tag=f"xr{b}")
                        nc.vector.tensor_copy(xl, xb[:, :, ds(i_sv, 1)])
                        nc.vector.tensor_copy(xr, xb[:, :, ds(j_sv, 1)])
                        nc.tensor.matmul(pcv, al_sb, xl, start=True, stop=False)
                        nc.tensor.matmul(pcv, ar_sb, xr, start=False, stop=True)
                    nc.vector.tensor_scalar(xb[:, :, slot:slot + 1], pcv,
                                            bias_sb, None, Alu.add)
                    if last:
                        continue
                    nc.scalar.activation(xbr[:, :, slot:slot + 1], pcv,
                                         ACT_IDENT, bias=bias_sb)
                    nc.scalar.activation(mstk, pcv, ACT_IDENT, bias=bias_sb)

                    # dots of the new slot vs all slots: 16 cf-chunks per
                    # f32r matmul (1 cycle/row at N>=256)
                    pd = psp.tile([16, 512], dt.float32, tag=f"pd{b}")
                    for t_ in range(CF // 16):
                        cf0 = 16 * t_
                        nc.tensor.matmul(pd, mstk[:, cf0:cf0 + 16],
                                         xbrF[:, 32 * cf0:32 * cf0 + 512],
                                         start=(t_ == 0), stop=(t_ == CF // 16 - 1))
                    pdS = sp.tile([16, 512], dt.float32, tag=f"pdS{b}")
                    nc.scalar.activation(pdS, pd, ACT_IDENT)

                    # diag-block extraction, accumulated in PSUM
                    pdd = psp.tile([1, M], dt.float32, tag=f"pdd{b}")
                    for g in range(16):
                        nc.tensor.matmul(pdd, id64[0:16, g:g + 1],
                                         pdS[:, g * 32:g * 32 + M],
                                         start=(g == 0), stop=(g == 15))

                    # distance updates
                    nc.vector.tensor_copy(sq[0:1, M * b + slot:M * b + slot + 1],
                                          pdd[0:1, slot:slot + 1])
                    t = sp.tile([1, M], dt.float32, tag=f"tups{b}")
                    nc.vector.tensor_scalar(t, pdd[0:1, :], 2.0,
                                            sq[0:1, M * b + slot:M * b + slot + 1],
                                            Alu.mult, Alu.subtract)
                    nc.vector.tensor_sub(t, t, sq[0:1, M * b:M * b + M])
                    ndrow = sp.tile([1, M], dt.float32, tag=f"ndrow{b}")
                    nc.vector.tensor_add(ndrow, t, pmaskF[0:1, M * b:M * b + M])
                    nc.vector.tensor_copy(v4[0:1, b, slot, 0:M], ndrow)
                    nc.vector.tensor_copy(v4[0:1, b, 0:M, slot:slot + 1], ndrow)
                    nc.vector.memset(v4[0:1, b, slot, slot:slot + 1], NEG)

            # ---- output: final slot of each batch element (staged contiguous) ----
            ystage = sp.tile([128, 2, CF], dt.float32, tag="ystage")
            for b in (0, 1):
                nc.vector.tensor_copy(ystage[:, b:b + 1, :], xbs[b][:, :, M - 1:M])
            nc.sync.dma_start(y[:], ystage)

    nc.finalize()
    return nc


_PROGRAM = None


def _get_program():
    global _PROGRAM
    if _PROGRAM is None:
        _PROGRAM = _build_program()
    return _PROGRAM


def _host_inputs(x: np.ndarray, conv_w: np.ndarray, conv_b: np.ndarray):
    B = x.shape[0]
    per = B // N_CORES
    assert per == 2
    A_l = _build_A(np.asarray(conv_w, np.float32)[0, 0])
    A_r = _build_A(np.asarray(conv_w, np.float32)[0, 1])

    def blkT(A):
        z = np.zeros((64, 64), np.float32)
        return np.ascontiguousarray(np.block([[A, z], [z, A]]).T.astype(np.float32))

    consts = dict(
        alT=blkT(A_l),
        arT=blkT(A_r),
        biasd=np.full((128, 1), np.float32(np.asarray(conv_b)[0]), np.float32),
        id64d=np.eye(64, dtype=np.float32),
    )
    # x (B,16,512,8,8) -> per-core per-batch (128, CF, M), unborn slots zero:
    # partition = (c%2)*64 + h*8+w ; cf = c//2
    xx = np.asarray(x, np.float32).reshape(B, N0, CF, 2, 64)
    xx = xx.transpose(3, 4, 2, 0, 1)                  # (2, 64, CF, B, N0)
    xx = np.ascontiguousarray(xx.reshape(128, CF, B, N0))
    shards = []
    for c in range(N_CORES):
        full = np.zeros((128, 2, CF, M), np.float32)
        for b in (0, 1):
            full[:, b, :, 0:N0] = xx[:, :, c * per + b, :]
        shards.append(full.reshape(128, 2 * CF * M))
    return shards, consts


def _host_output(y_cores) -> np.ndarray:
    ys = np.stack(y_cores, axis=1)                    # (128, ncores, 2, 256)
    ys = ys.reshape(2, 64, N_CORES * 2, CF)           # (cp, p, b, cf)
    out = ys.transpose(2, 3, 0, 1)                    # (b, cf, cp, p)
    return np.ascontiguousarray(out.reshape(N_CORES * 2, 512, 8, 8))


def kernel(x: np.ndarray, conv_w: np.ndarray, conv_b: np.ndarray, _res_hook=None) -> np.ndarray:
    nc = _get_program()
    shards, consts = _host_inputs(x, conv_w, conv_b)
    in_maps = [{"xpf": shards[c], **consts} for c in range(N_CORES)]
    res = run_bass_kernel_spmd(nc, in_maps, list(range(N_CORES)))
    if _res_hook is not None:
        _res_hook(res)
    return _host_output([res.results[c]["y"] for c in range(N_CORES)])


if __name__ == "__main__":
    rng = np.random.default_rng(0)
    x = rng.standard_normal((16, 16, 512, 8, 8)).astype(np.float32)
    w = (rng.standard_normal((1, 2, 7, 7)) * 0.1).astype(np.float32)
    b = (rng.standard_normal((1,)) * 0.1).astype(np.float32)
    out = kernel(x, w, b)
    print("kernel ran, out shape", out.shape)


# revision 31
# speedup vs baseline: 1.0006x; 1.0006x over previous
"""Trainium2 Bass kernel for agglomerative pairwise conv-merging (retrieval_knn).

Problem: x (16,16,512,8,8) f32. Per batch element: n=16 slots, 15 merge
iterations; each picks the argmin pair of the masked pairwise squared-distance
matrix, merges the pair via a 2->1-channel 7x7 conv (pad 3), writes the result
to a new slot. Output = final slot per batch element.

Strategy: pure data parallelism, 2 batch elements per core on 8 cores; the
whole per-core working set lives in SBUF so HBM traffic is one input read +
one output write. The two batch elements share no state, so the Tile
scheduler can overlap one batch's TensorE phase with the other's VectorE
phase. Per merge iteration, fully on-chip:
  - cf-major slot layout (128, cf, s) per batch: partition = (c%2)*64 + h*8+w,
    cf = c//2. An f32r (FP22) shadow feeds all distance matmuls at
    1 cycle/row; the fp32 master feeds the conv and the output.
  - conv == per-channel 64x64 Toeplitz matmul (pad-3 7x7 conv on 8x8 is a
    dense 64x64 spatial operator); block-diag 128x128 on the PE, reading the
    merging pair in place through register-indexed dynamic APs.
  - negated masked distances live free-form on partition 0 as 32-padded
    matrices, so argmin is a flat max/max_index and row/col kills are
    free-dim memsets with register-derived offsets (i = idx>>5, j = idx&31).
  - new-slot dots vs all slots: 16 cf-chunks per f32r matmul (N=496,
    16 matmuls per batch), then diag-block extraction matmuls accumulated
    in PSUM.
"""

import sys

sys.path.insert(0, "/opt/trn_rl_repo")

import numpy as np

import concourse.bacc as bacc
import concourse.bass as bass
import concourse.mybir as mybir
from concourse.bass_utils import run_bass_kernel_spmd
from concourse.tile import TileContext

dt = mybir.dt
Alu = mybir.AluOpType
DVE = mybir.EngineType.DVE
PE = mybir.EngineType.PE
ACT_IDENT = mybir.ActivationFunctionType.Identity
ds = bass.ds

N_CORES = 8
N0 = 16          # initial slots per batch element
M = 31           # total slots (2*16-1)
CF = 256         # channel pairs (512 channels / 2)
NEG = -1e30
USE_DYN_CONV = True


def _build_A(w7: np.ndarray) -> np.ndarray:
    """(7,7) conv weights -> dense (64,64) operator on the 8x8 spatial dim."""
    A = np.zeros((64, 64), np.float32)
    ky, kx = np.meshgrid(np.arange(7), np.arange(7), indexing="ij")
    for y in range(8):
        for x in range(8):
            yy, xx = y + ky - 3, x + kx - 3
            ok = (yy >= 0) & (yy < 8) & (xx >= 0) & (xx < 8)
            A[y * 8 + x, (yy * 8 + xx)[ok]] = w7[ok]
    return A


def _build_program():
    nc = bacc.Bacc(None, target_bir_lowering=False, debug=False)
    xpf = nc.declare_dram_parameter("xpf", [128, 2 * CF * M], dt.float32, isOutput=False)
    alT = nc.declare_dram_parameter("alT", [128, 128], dt.float32, isOutput=False)
    arT = nc.declare_dram_parameter("arT", [128, 128], dt.float32, isOutput=False)
    biasd = nc.declare_dram_parameter("biasd", [128, 1], dt.float32, isOutput=False)
    id64d = nc.declare_dram_parameter("id64d", [64, 64], dt.float32, isOutput=False)
    y = nc.declare_dram_parameter("y", [128, 2, CF], dt.float32, isOutput=True)

    with TileContext(nc) as tc:
        with (
            tc.tile_pool(name="persist", bufs=1) as pp,
            tc.tile_pool(name="scratch", bufs=6) as sp,
            tc.tile_pool(name="ps0", bufs=1, space="PSUM") as ps0,
            tc.tile_pool(name="ps1", bufs=1, space="PSUM") as ps1,
        ):
            # ---- persistent state (independent per batch) ----
            xbs = [pp.tile([128, CF, M], dt.float32, tag=f"xb{b}", name=f"xb{b}") for b in (0, 1)]
            xbrs = [pp.tile([128, CF, 32], dt.float32r, tag=f"xbr{b}", name=f"xbr{b}") for b in (0, 1)]
            mstks = [pp.tile([128, CF], dt.float32r, tag=f"mstk{b}", name=f"mstk{b}") for b in (0, 1)]
            ndmat = pp.tile([1, 2048], dt.float32, tag="ndmat")
            pmaskF = pp.tile([1, 2 * M], dt.float32, tag="pmaskF")
            sq = pp.tile([1, 2 * M], dt.float32, tag="sq")
            gflat = pp.tile([1, 2, 16, 16], dt.float32, tag="gflat")
            al_sb = pp.tile([128, 128], dt.float32, tag="al")
            ar_sb = pp.tile([128, 128], dt.float32, tag="ar")
            bias_sb = pp.tile([128, 1], dt.float32, tag="bias")
            id64 = pp.tile([64, 64], dt.float32, tag="id64")

            xbrFs = [t.rearrange("p f s -> p (f s)") for t in xbrs]
            v4 = ndmat.rearrange("p (b r c) -> p b r c", b=2, r=32)

            # ---- loads ----
            nc.sync.dma_start(al_sb, alT[:])
            nc.sync.dma_start(ar_sb, arT[:])
            nc.sync.dma_start(bias_sb, biasd[:])
            nc.sync.dma_start(id64, id64d[:])
            for b in (0, 1):
                nc.sync.dma_start(xbs[b].rearrange("p f s -> p (f s)"),
                                  xpf[:, b * CF * M:(b + 1) * CF * M])
                nc.vector.tensor_copy(xbrs[b][:, :, 0:M], xbs[b])
                nc.vector.tensor_copy(xbrs[b][:, :, M:32], xbs[b][:, :, 0:1])
            nc.vector.memset(ndmat, NEG)
            nc.vector.memset(pmaskF, NEG)
            nc.vector.memset(sq, 0.0)
            for b in (0, 1):
                nc.vector.memset(pmaskF[0:1, M * b:M * b + N0], 0.0)

            # ---- initial Gram per batch (f32r; FP22 is plenty for argmin) ----
            for b in (0, 1):
                psp = ps0 if b == 0 else ps1
                pG = psp.tile([32, 32], dt.float32, tag=f"pd{b}", bufs=2)
                for cf in range(CF):
                    src = xbrs[b][:, cf, :]
                    nc.tensor.matmul(pG, src, src,
                                     start=(cf == 0), stop=(cf == CF - 1))
                gsb = sp.tile([32, 32], dt.float32, tag=f"gsb{b}")
                nc.vector.tensor_copy(gsb, pG)
                # flatten the 16 needed Gram rows to free-form
                for k in range(16):
                    pr = psp.tile([1, 16], dt.float32, tag=f"pdd{b}")
                    nc.tensor.matmul(pr, id64[0:32, k:k + 1], gsb[:, 0:16],
                                     start=True, stop=True)
                    nc.vector.tensor_copy(gflat[0:1, b, k, :], pr)
                for k in range(16):
                    nc.vector.tensor_copy(sq[0:1, M * b + k:M * b + k + 1],
                                          gflat[0:1, b, k, k:k + 1])
                # initial negated distances: nd[k,s] = 2 G[k,s] - sq_k - sq_s
                for k in range(16):
                    t = sp.tile([1, 16], dt.float32, tag=f"tinit{b}")
                    nc.vector.tensor_scalar(
                        t, gflat[0:1, b, k, :], 2.0,
                        sq[0:1, M * b + k:M * b + k + 1], Alu.mult, Alu.subtract)
                    nc.vector.tensor_sub(v4[0:1, b, k, 0:16], t,
                                         sq[0:1, M * b:M * b + 16])
                    nc.vector.memset(v4[0:1, b, k, k:k + 1], NEG)

            # ---- 15 merge iterations ----
            for it in range(M - N0):
                slot = N0 + it
                last = it == M - N0 - 1
                for b in (0, 1):
                    psp = ps0 if b == 0 else ps1
                    xb, xbr, mstk, xbrF = xbs[b], xbrs[b], mstks[b], xbrFs[b]

                    # argmin over masked negated distances
                    mx8 = sp.tile([1, 8], dt.float32, tag=f"mx8_{b}")
                    ix8 = sp.tile([1, 8], dt.uint32, tag=f"ix8_{b}")
                    nd_b = ndmat[0:1, 1024 * b:1024 * (b + 1)]
                    nc.vector.max(mx8, nd_b)
                    nc.vector.max_index(ix8, mx8, nd_b)
                    engines = (DVE, PE) if USE_DYN_CONV else (DVE,)
                    idx_sv = nc.values_load(ix8[0:1, 0:1], engines=engines,
                                            min_val=0, max_val=1023,
                                            skip_runtime_bounds_check=True)
                    ri = nc.alloc_registers(f"ri{it}_{b}", engines=engines)
                    nc.regs_alu(ri, idx_sv, 5, Alu.logical_shift_right)
                    i_sv = nc.snap(ri, donate=True, min_val=0, max_val=M - 1)
                    rj = nc.alloc_registers(f"rj{it}_{b}", engines=engines)
                    nc.regs_alu(rj, idx_sv, 31, Alu.bitwise_and)
                    j_sv = nc.snap(rj, donate=True, min_val=0, max_val=M - 1)

                    # alive-mask + row/col kills (before the new row is built)
                    pmb = pmaskF[0:1, M * b:M * b + M]
                    nc.vector.memset(pmb[0:1, ds(i_sv, 1)], NEG)
                    nc.vector.memset(pmb[0:1, ds(j_sv, 1)], NEG)
                    if not last:
                        nc.vector.memset(v4[0:1, b, ds(i_sv, 1), :], NEG)
                        nc.vector.memset(v4[0:1, b, ds(j_sv, 1), :], NEG)
                        nc.vector.memset(v4[0:1, b, 0:M, ds(i_sv, 1)], NEG)
                        nc.vector.memset(v4[0:1, b, 0:M, ds(j_sv, 1)], NEG)
                        nc.vector.memset(pmaskF[0:1, M * b + slot:M * b + slot + 1], 0.0)

                    # conv merge m = A_l @ x_i + A_r @ x_j + bias
                    pcv = psp.tile([128, CF], dt.float32, tag=f"pcv{b}")
                    if USE_DYN_CONV:
                        nc.tensor.matmul(pcv, al_sb, xb[:, :, ds(i_sv, 1)],
                                         start=True, stop=False)
                        nc.tensor.matmul(pcv, ar_sb, xb[:, :, ds(j_sv, 1)],
                                         start=False, stop=True)
                    else:
                        xl = sp.tile([128, CF], dt.float32, tag=f"xl{b}")
                        xr = sp.tile([128, CF], dt.float32, tag=f"xr{b}")
                        nc.vector.tensor_copy(xl, xb[:, :, ds(i_sv, 1)])
                        nc.vector.tensor_copy(xr, xb[:, :, ds(j_sv, 1)])
                        nc.tensor.matmul(pcv, al_sb, xl, start=True, stop=False)
                        nc.tensor.matmul(pcv, ar_sb, xr, start=False, stop=True)
                    nc.vector.tensor_scalar(xb[:, :, slot:slot + 1], pcv,
                                            bias_sb, None, Alu.add)
                    if last:
                        continue
                    nc.scalar.activation(xbr[:, :, slot:slot + 1], pcv,
                                         ACT_IDENT, bias=bias_sb)
                    nc.scalar.activation(mstk, pcv, ACT_IDENT, bias=bias_sb)

                    # dots of the new slot vs all slots: 16 cf-chunks per
                    # f32r matmul (1 cycle/row at N>=256)
                    pd = psp.tile([16, 512], dt.float32, tag=f"pd{b}", bufs=2)
                    for t_ in range(CF // 16):
                        cf0 = 16 * t_
                        nc.tensor.matmul(pd, mstk[:, cf0:cf0 + 16],
                                         xbrF[:, 32 * cf0:32 * cf0 + 512],
                                         start=(t_ == 0), stop=(t_ == CF // 16 - 1))
                    pdS = sp.tile([16, 512], dt.float32, tag=f"pdS{b}")
                    nc.scalar.activation(pdS, pd, ACT_IDENT)

                    # diag-block extraction, accumulated in PSUM
                    pdd = psp.tile([1, M], dt.float32, tag=f"pdd{b}")
                    for g in range(16):
                        nc.tensor.matmul(pdd, id64[0:16, g:g + 1],
                                         pdS[:, g * 32:g * 32 + M],
                                         start=(g == 0), stop=(g == 15))

                    # distance updates
                    nc.vector.tensor_copy(sq[0:1, M * b + slot:M * b + slot + 1],
                                          pdd[0:1, slot:slot + 1])
                    t = sp.tile([1, M], dt.float32, tag=f"tups{b}")
                    nc.vector.tensor_scalar(t, pdd[0:1, :], 2.0,
                                            sq[0:1, M * b + slot:M * b + slot + 1],
                                            Alu.mult, Alu.subtract)
                    nc.vector.tensor_sub(t, t, sq[0:1, M * b:M * b + M])
                    ndrow = sp.tile([1, M], dt.float32, tag=f"ndrow{b}")
                    nc.vector.tensor_add(ndrow, t, pmaskF[0:1, M * b:M * b + M])
                    nc.vector.tensor_copy(v4[0:1, b, slot, 0:M], ndrow)
                    nc.vector.tensor_copy(v4[0:1, b, 0:M, slot:slot + 1], ndrow)
                    nc.vector.memset(v4[0:1, b, slot, slot:slot + 1], NEG)

            # ---- output: final slot of each batch element (staged contiguous) ----
            ystage = sp.tile([128, 2, CF], dt.float32, tag="ystage")
            for b in (0, 1):
                nc.vector.tensor_copy(ystage[:, b:b + 1, :], xbs[b][:, :, M - 1:M])
            nc.sync.dma_start(y[:], ystage)

    nc.finalize()
    return nc


_PROGRAM = None


def _get_program():
    global _PROGRAM
    if _PROGRAM is None:
        _PROGRAM = _build_program()
    return _PROGRAM


def _host_inputs(x: np.ndarray, conv_w: np.ndarray, conv_b: np.ndarray):
    B = x.shape[0]
    per = B // N_CORES
    assert per == 2
    A_l = _build_A(np.asarray(conv_w, np.float32)[0, 0])
    A_r = _build_A(np.asarray(conv_w, np.float32)[0, 1])

    def blkT(A):
        z = np.zeros((64, 64), np.float32)
        return np.ascontiguousarray(np.block([[A, z], [z, A]]).T.astype(np.float32))

    consts = dict(
        alT=blkT(A_l),
        arT=blkT(A_r),
        biasd=np.full((128, 1), np.float32(np.asarray(conv_b)[0]), np.float32),
        id64d=np.eye(64, dtype=np.float32),
    )
    # x (B,16,512,8,8) -> per-core per-batch (128, CF, M), unborn slots zero:
    # partition = (c%2)*64 + h*8+w ; cf = c//2
    xx = np.asarray(x, np.float32).reshape(B, N0, CF, 2, 64)
    xx = xx.transpose(3, 4, 2, 0, 1)                  # (2, 64, CF, B, N0)
    xx = np.ascontiguousarray(xx.reshape(128, CF, B, N0))
    shards = []
    for c in range(N_CORES):
        full = np.zeros((128, 2, CF, M), np.float32)
        for b in (0, 1):
            full[:, b, :, 0:N0] = xx[:, :, c * per + b, :]
        shards.append(full.reshape(128, 2 * CF * M))
    return shards, consts


def _host_output(y_cores) -> np.ndarray:
    ys = np.stack(y_cores, axis=1)                    # (128, ncores, 2, 256)
    ys = ys.reshape(2, 64, N_CORES * 2, CF)           # (cp, p, b, cf)
    out = ys.transpose(2, 3, 0, 1)                    # (b, cf, cp, p)
    return np.ascontiguousarray(out.reshape(N_CORES * 2, 512, 8, 8))


def kernel(x: np.ndarray, conv_w: np.ndarray, conv_b: np.ndarray, _res_hook=None) -> np.ndarray:
    nc = _get_program()
    shards, consts = _host_inputs(x, conv_w, conv_b)
    in_maps = [{"xpf": shards[c], **consts} for c in range(N_CORES)]
    res = run_bass_kernel_spmd(nc, in_maps, list(range(N_CORES)))
    if _res_hook is not None:
        _res_hook(res)
    return _host_output([res.results[c]["y"] for c in range(N_CORES)])


if __name__ == "__main__":
    rng = np.random.default_rng(0)
    x = rng.standard_normal((16, 16, 512, 8, 8)).astype(np.float32)
    w = (rng.standard_normal((1, 2, 7, 7)) * 0.1).astype(np.float32)
    b = (rng.standard_normal((1,)) * 0.1).astype(np.float32)
    out = kernel(x, w, b)
    print("kernel ran, out shape", out.shape)
